# revision 21
# baseline (speedup 1.0000x reference)
"""Trainium2 Bass kernel for nn_DiagonalLayer (per-gene weighted feature sum).

out[b, g] = sum_f x[b, 3g+f] * w[3g+f] + bias[g]

Sharding: data-parallel over the batch dim — 4096 rows split as 512 rows on
each of the 8 NeuronCores; w/bias replicated (tiny). Output gathered by
concatenation along batch.

Self-contained: hardcodes shapes; only imports the concourse toolchain.
"""

import sys

import numpy as np

if "/opt/trn_rl_repo" not in sys.path:
    sys.path.insert(0, "/opt/trn_rl_repo")

B = 4096
GF = 27687
G = 9229
F = 3
NCORES = 8
BSH = B // NCORES  # 512 batch rows per core
PT = 128  # SBUF partitions
NT = BSH // PT  # 4 batch tiles per core
GC = 2308  # genes per chunk (v1)

# v2 knobs
V2_GC = 1536  # genes per chunk
V2_SPLIT = 0.68  # fraction of genes computed on DVE (rest on GpSimd)
V2_DVE_METHOD = "reduce"  # "adds" (strided) or "reduce"

import os as _os

VARIANT = _os.environ.get("KERNEL_VARIANT", "v2")

_cached_nc = None


def _gene_chunks(gc_size=GC):
    chunks = []
    c0 = 0
    while c0 < G:
        gc = min(gc_size, G - c0)
        chunks.append((c0, gc))
        c0 += gc
    return chunks


def _build_nc():
    import concourse.bacc as bacc
    import concourse.mybir as mybir
    import concourse.tile as tile

    f32 = mybir.dt.float32
    nc = bacc.Bacc(
        "TRN2", target_bir_lowering=False, debug=False, num_devices=NCORES
    )
    x = nc.dram_tensor("x", [BSH, GF], f32, kind="ExternalInput")
    w = nc.dram_tensor("w", [GF], f32, kind="ExternalInput")
    bias = nc.dram_tensor("bias", [G], f32, kind="ExternalInput")
    out = nc.dram_tensor("out", [BSH, G], f32, kind="ExternalOutput")

    if VARIANT == "v1":
        _emit_v1(nc, tile, mybir, f32, x, w, bias, out)
    else:
        _emit_v2(nc, tile, mybir, f32, x, w, bias, out)
    if not nc.is_finalized():
        nc.finalize()
    return nc


def _emit_v2(nc, tile, mybir, f32, x, w, bias, out):
    """Gene-split variant: per (chunk, batch-tile) iteration, DVE computes the
    first V2_SPLIT fraction of genes (mult + strided segment adds + bias) while
    GpSimd computes the rest. w/bias are broadcast across partitions via
    TensorE ones-matmul + ScalarE PSUM->SBUF copy, keeping DMA rings free."""
    with tile.TileContext(nc) as tc:
        with (
            tc.tile_pool(name="const", bufs=1) as const_pool,
            tc.tile_pool(name="wrow", bufs=2) as row_pool,
            tc.tile_pool(name="psum", bufs=6, space="PSUM") as psum_pool,
            tc.tile_pool(name="wb", bufs=3) as wb_pool,
            tc.tile_pool(name="bb", bufs=3) as bb_pool,
            tc.tile_pool(name="xa", bufs=3) as xa_pool,
            tc.tile_pool(name="xb", bufs=3) as xb_pool,
            tc.tile_pool(name="oa", bufs=4) as oa_pool,
            tc.tile_pool(name="ob", bufs=4) as ob_pool,
        ):
            ones = const_pool.tile([1, PT], f32, tag="ones")
            nc.vector.memset(ones[:, :], 1.0)

            ROW = 1024  # row-load granularity (two 512-wide matmuls per row)

            def bcast(dst, src_dram, off, n_total):
                # dst[p, j] = src_dram[off + j] for all 128 partitions
                for o in range(0, n_total, ROW):
                    n = min(ROW, n_total - o)
                    row = row_pool.tile([1, ROW], f32, tag="wrow")
                    nc.sync.dma_start(
                        out=row[:1, :n], in_=src_dram[None, off + o : off + o + n]
                    )
                    for o2 in range(0, n, 512):
                        n2 = min(512, n - o2)
                        ps = psum_pool.tile([PT, 512], f32, tag="ps")
                        nc.tensor.matmul(
                            ps[:, :n2], ones[:1, :], row[:1, o2 : o2 + n2]
                        )
                        nc.scalar.copy(dst[:, o + o2 : o + o2 + n2], ps[:, :n2])

            chunks = _gene_chunks(V2_GC)
            # runt chunk first: its broadcast chain is short, cutting the
            # serial ramp before the first compute op
            chunks = chunks[-1:] + chunks[:-1]

            def bcast_chunk(c0, gc):
                wbt = wb_pool.tile([PT, F * gc], f32, tag="wb")
                bcast(wbt, w, F * c0, F * gc)
                bbt = bb_pool.tile([PT, gc], f32, tag="bb")
                bcast(bbt, bias, c0, gc)
                return wbt, bbt

            pending = [bcast_chunk(*chunks[0])]
            if len(chunks) > 1:
                pending.append(bcast_chunk(*chunks[1]))
            for ci, (c0, gc) in enumerate(chunks):
                wbt, bbt = pending.pop(0)
                s = int(round(gc * V2_SPLIT))
                nb = gc - s

                for t in range(NT):
                    rows = slice(t * PT, (t + 1) * PT)
                    # --- DVE range: genes [c0, c0+s) ---
                    xa_t = xa_pool.tile([PT, F * s], f32, tag="xa")
                    nc.sync.dma_start(
                        out=xa_t[:, :], in_=x[rows, F * c0 : F * (c0 + s)]
                    )
                    oa_t = oa_pool.tile([PT, s], f32, tag="oa")
                    nc.vector.tensor_mul(xa_t[:, :], xa_t[:, :], wbt[:, : F * s])
                    y3 = xa_t[:, :].rearrange("p (g f) -> p g f", f=F)
                    if V2_DVE_METHOD == "adds":
                        nc.vector.tensor_add(oa_t[:, :], y3[:, :, 0], y3[:, :, 1])
                        nc.vector.tensor_add(oa_t[:, :], oa_t[:, :], y3[:, :, 2])
                    else:
                        nc.vector.reduce_sum(
                            oa_t[:, :], y3, axis=mybir.AxisListType.X
                        )
                    nc.vector.tensor_add(oa_t[:, :], oa_t[:, :], bbt[:, :s])
                    # stores go on the ACT HWDGE queue so the SP queue (x
                    # loads) never blocks behind a compute-dependent store
                    nc.scalar.dma_start(out=out[rows, c0 : c0 + s], in_=oa_t[:, :])

                    # --- GpSimd range: genes [c0+s, c0+gc) ---
                    xb_t = xb_pool.tile([PT, F * nb], f32, tag="xb")
                    nc.sync.dma_start(
                        out=xb_t[:, :], in_=x[rows, F * (c0 + s) : F * (c0 + gc)]
                    )
                    ob_t = ob_pool.tile([PT, nb], f32, tag="ob")
                    nc.gpsimd.tensor_mul(
                        xb_t[:, :], xb_t[:, :], wbt[:, F * s : F * gc]
                    )
                    z3 = xb_t[:, :].rearrange("p (g f) -> p g f", f=F)
                    nc.gpsimd.tensor_add(ob_t[:, :], z3[:, :, 0], z3[:, :, 1])
                    nc.gpsimd.tensor_add(ob_t[:, :], ob_t[:, :], z3[:, :, 2])
                    nc.gpsimd.tensor_add(ob_t[:, :], ob_t[:, :], bbt[:, s:gc])
                    nc.scalar.dma_start(
                        out=out[rows, c0 + s : c0 + gc], in_=ob_t[:, :]
                    )

                    if t == 0 and ci + 2 < len(chunks):
                        # keep two chunk broadcasts in flight (bufs=3 pools)
                        pending.append(bcast_chunk(*chunks[ci + 2]))


def _emit_v1(nc, tile, mybir, f32, x, w, bias, out):
    with tile.TileContext(nc) as tc:
        with (
            tc.tile_pool(name="wb", bufs=2) as wb_pool,
            tc.tile_pool(name="bb", bufs=2) as bb_pool,
            tc.tile_pool(name="xc", bufs=3) as x_pool,
            tc.tile_pool(name="oc", bufs=3) as o_pool,
        ):
            for c0, gc in _gene_chunks():
                wbt = wb_pool.tile([PT, F * gc], f32, tag="wb")
                nc.sync.dma_start(
                    out=wbt[:1, :], in_=w[None, F * c0 : F * (c0 + gc)]
                )
                nc.gpsimd.partition_broadcast(wbt[:, :], wbt[:1, :])

                bbt = bb_pool.tile([PT, gc], f32, tag="bb")
                nc.sync.dma_start(out=bbt[:1, :], in_=bias[None, c0 : c0 + gc])
                nc.gpsimd.partition_broadcast(bbt[:, :], bbt[:1, :])

                for t in range(NT):
                    xc = x_pool.tile([PT, F * gc], f32, tag="xc")
                    nc.sync.dma_start(
                        out=xc[:, :],
                        in_=x[t * PT : (t + 1) * PT, F * c0 : F * (c0 + gc)],
                    )
                    nc.vector.tensor_mul(xc[:, :], xc[:, :], wbt[:, :])
                    oc = o_pool.tile([PT, gc], f32, tag="oc")
                    x3 = xc[:, :].rearrange("p (g f) -> p g f", f=F)
                    nc.vector.reduce_sum(oc[:, :], x3, axis=mybir.AxisListType.X)
                    nc.vector.tensor_add(oc[:, :], oc[:, :], bbt[:, :])
                    nc.sync.dma_start(
                        out=out[t * PT : (t + 1) * PT, c0 : c0 + gc], in_=oc[:, :]
                    )


def _get_nc():
    global _cached_nc
    if _cached_nc is None:
        _cached_nc = _build_nc()
    return _cached_nc


def run(x, weights, bias, trace=False, tmpdir=None):
    from concourse.bass_utils import run_bass_kernel_spmd

    x = np.ascontiguousarray(np.asarray(x, dtype=np.float32))
    weights = np.ascontiguousarray(np.asarray(weights, dtype=np.float32))
    bias_np = np.ascontiguousarray(np.asarray(bias, dtype=np.float32))

    nc = _get_nc()
    in_maps = [
        {
            "x": np.ascontiguousarray(x[c * BSH : (c + 1) * BSH]),
            "w": weights,
            "bias": bias_np,
        }
        for c in range(NCORES)
    ]
    try:
        res = run_bass_kernel_spmd(
            nc, in_maps, list(range(NCORES)), trace=trace, tmpdir=tmpdir
        )
    except Exception:
        # transient NRT device errors (e.g. NRT_EXEC_UNIT_UNRECOVERABLE after
        # a wedged run) usually clear on retry
        res = run_bass_kernel_spmd(
            nc, in_maps, list(range(NCORES)), trace=trace, tmpdir=tmpdir
        )
    outs = [res.results[c]["out"] for c in range(NCORES)]
    full = np.concatenate(outs, axis=0)
    return full, res


def kernel(x, weights, bias):
    full, _ = run(x, weights, bias, trace=False)
    return full
